# revision 18
# baseline (speedup 1.0000x reference)
"""Trainium2 Bass kernel for CrossAttention with LoRA.

Strategy: data-parallel over batch (B=8 -> 8 NeuronCores, one batch element
per core). No collectives. Per-core compute is a fully fused cross-attention,
restructured (vs the previous version) around three measured bottlenecks:
DVE (vector) saturation, a long serial tail, and rowsum row-copies.

  kT [C,S] = (Wf[:C].T row-tile col-slices) @ fT
  v  [S,C] = fT col-slices as lhsT @ Wf[C:].T          (natural layout)
  qT [C,T] = (Wq.T row-tile col-slices) @ xT           (x passed pre-transposed)
  per half hf (t in [hf*512,(hf+1)*512)):
    per head h: s[S,Thalf] = kT_h.T-slices @ qT_h      (K=D=64)
                e = exp(s/8) (* causal mask, first half only)
                rowsums accumulate into rz[16,512] via indicator matmuls
                  (lhsT = ones-column-h [128,16]) -- no [1,T] row copies
    attv: pair-packed psum [128,512] (even head rows 0:64, odd 64:128 via
                  tile_position), ONE drain per (pair, half)
    normalize: reciprocal_approx_fast + f32r selection-matrix broadcast
                  matmul, yTr *= rb (DVE/GpSimd split)
    oproj m-tiles inside this half start as soon as the half is normalized,
                  overlapping the other half's attention.
Engine budget: exp stream on Scalar; PSUM drains split DVE/GpSimd; ost
(out staging) on Scalar/GpSimd; output stores split across both HWDGE
queues. Weight/input DMAs ordered by first use (wp last).
LoRA terms (rank 16) and biases fold into the same PSUM accumulation groups;
they are skipped at trace time when the corresponding host arrays are zero
(true for loralib-initialized B matrices and zero biases).
"""

import ml_dtypes
import numpy as np

import concourse.bass as bass  # noqa: F401  (bass types via bacc)
import concourse.mybir as mybir
import concourse.tile as tile
from concourse import bacc
from concourse.bass_utils import run_bass_kernel_spmd

B, T, S, C, H, D, R = 8, 1024, 256, 1024, 16, 64, 16
SCALING = 1.0 / 16.0
P = 128
KC = C // P  # 8 k-tiles over the embedding dim
MT = T // P  # 8 tiles over T
NHF = 2      # two T-halves of 512
HW = 512     # half width
F32 = mybir.dt.float32
F32R = mybir.dt.float32r
BF16 = mybir.dt.bfloat16
NPBF16 = ml_dtypes.bfloat16

_nc_cache: dict = {}


def _build(flags):
    has_lq, has_lf, has_lp, has_bq, has_bfk, has_bfv, has_bp = flags
    nc = bacc.Bacc("TRN2", target_bir_lowering=False, debug=False)

    xT = nc.declare_dram_parameter("xT", [C, T], BF16, isOutput=False)
    fT = nc.declare_dram_parameter("fT", [C, S], BF16, isOutput=False)
    WqT = nc.declare_dram_parameter("WqT", [C, C], BF16, isOutput=False)
    WfkT = nc.declare_dram_parameter("WfkT", [C, C], BF16, isOutput=False)
    WfvT = nc.declare_dram_parameter("WfvT", [C, C], BF16, isOutput=False)
    WpT = nc.declare_dram_parameter("WpT", [C, C], BF16, isOutput=False)
    mask = nc.declare_dram_parameter("mask", [P, 384], BF16, isOutput=False)
    Esel = nc.declare_dram_parameter("Esel", [H, C], F32R, isOutput=False)
    Hsel = nc.declare_dram_parameter("Hsel", [P, H * H], BF16, isOutput=False)
    if has_lq:
        AqT = nc.declare_dram_parameter("AqT", [C, R], BF16, isOutput=False)
        BqTs = nc.declare_dram_parameter("BqTs", [R, C], BF16, isOutput=False)
    if has_lf:
        AfT = nc.declare_dram_parameter("AfT", [C, R], BF16, isOutput=False)
        BfkTs = nc.declare_dram_parameter("BfkTs", [R, C], BF16, isOutput=False)
        BfvTs = nc.declare_dram_parameter("BfvTs", [R, C], BF16, isOutput=False)
    if has_lp:
        ApT = nc.declare_dram_parameter("ApT", [C, R], BF16, isOutput=False)
        BpTs = nc.declare_dram_parameter("BpTs", [R, C], BF16, isOutput=False)
    if has_bq:
        bq_pp = nc.declare_dram_parameter("bq_pp", [P, KC], F32, isOutput=False)
    if has_bfk:
        bfk_pp = nc.declare_dram_parameter("bfk_pp", [P, KC], F32, isOutput=False)
    if has_bfv:
        bfv_row = nc.declare_dram_parameter("bfv_row", [1, C], BF16, isOutput=False)
    if has_bp:
        bp_row = nc.declare_dram_parameter("bp_row", [1, C], BF16, isOutput=False)
    out = nc.declare_dram_parameter("out", [T, C], F32, isOutput=True)

    # 3D row-tile views for contiguous tiled DMA
    xT3 = xT.rearrange("(ko p) t -> ko p t", p=P)
    fT3 = fT.rearrange("(ko p) s -> ko p s", p=P)
    WqT3 = WqT.rearrange("(ko p) c -> ko p c", p=P)
    WfkT3 = WfkT.rearrange("(ko p) c -> ko p c", p=P)
    WfvT3 = WfvT.rearrange("(ko p) c -> ko p c", p=P)
    WpT3 = WpT.rearrange("(ko p) c -> ko p c", p=P)

    def c512(i):
        return slice(i * 512, (i + 1) * 512)

    def mP(m):
        return slice(m * P, (m + 1) * P)

    with tile.TileContext(nc) as tc:
        with (
            tc.tile_pool(name="big", bufs=8) as big,      # xT tiles
            tc.tile_pool(name="ypool", bufs=8) as ypool,  # yTr tiles
            tc.tile_pool(name="qpool", bufs=8) as qpool,  # qT tiles
            tc.tile_pool(name="wts", bufs=32) as wts,     # wfk/wfv/wq/wp row tiles
            tc.tile_pool(name="small", bufs=1) as small,  # long-lived small tiles
            tc.tile_pool(name="expp", bufs=8) as expp,    # per-head exp tiles
            tc.tile_pool(name="ostg", bufs=4) as ostg,    # out staging
            tc.tile_pool(name="psA", bufs=5, space="PSUM") as psA,  # score tiles
            tc.tile_pool(name="psY", bufs=2, space="PSUM") as psY,  # qproj + attv
            tc.tile_pool(name="psR", bufs=1, space="PSUM") as psR,  # rowsum tiles
        ):
            # ---- SP queue: mask/hsel first (tiny), then k/v operands, wp last --
            mask_sb = small.tile([P, 384], BF16, tag="mask", name="mask_sb")
            nc.sync.dma_start(mask_sb[:], mask[:, :])
            hsel_sb = small.tile([P, H, H], BF16, tag="hsel", name="hsel_sb")
            nc.sync.dma_start(hsel_sb[:], Hsel.rearrange("p (h j) -> p h j", h=H))
            fTs = [small.tile([P, S], BF16, tag=f"fT{k}", name=f"fT{k}") for k in range(KC)]
            for k in range(KC):
                nc.sync.dma_start(fTs[k][:], fT3[k])
            wfk = [wts.tile([P, C], BF16, tag="wts", name=f"wfk{k}") for k in range(KC)]
            for k in range(KC):
                nc.sync.dma_start(wfk[k][:], WfkT3[k])
            wfv = [wts.tile([P, C], BF16, tag="wts", name=f"wfv{k}") for k in range(KC)]
            for k in range(KC):
                nc.sync.dma_start(wfv[k][:], WfvT3[k])
            wp = [wts.tile([P, C], BF16, tag="wts", name=f"wp{k}") for k in range(KC)]
            for k in range(KC):
                nc.sync.dma_start(wp[k][:], WpT3[k])
            # ---- Act queue: q-projection operands, then esel -------------------
            xTs = [big.tile([P, T], BF16, tag="big", name=f"xT{k}") for k in range(KC)]
            for k in range(KC):
                nc.scalar.dma_start(xTs[k][:], xT3[k])
            wq = [wts.tile([P, C], BF16, tag="wts", name=f"wq{k}") for k in range(KC)]
            for k in range(KC):
                nc.scalar.dma_start(wq[k][:], WqT3[k])
            esel_sb = small.tile([H, C], F32R, tag="esel", name="esel_sb")
            nc.scalar.dma_start(esel_sb[:], Esel[:, :])
            # ---- small conditional loads (SP) ----------------------------------
            if has_lq:
                aq_sb = small.tile([P, KC, R], BF16, tag="aq", name="aq_sb")
                nc.sync.dma_start(aq_sb[:], AqT.rearrange("(ko p) r -> p ko r", p=P))
                bqs_sb = small.tile([R, C], BF16, tag="bqs", name="bqs_sb")
                nc.sync.dma_start(bqs_sb[:], BqTs[:, :])
            if has_lf:
                af_sb = small.tile([P, KC, R], BF16, tag="af", name="af_sb")
                nc.sync.dma_start(af_sb[:], AfT.rearrange("(ko p) r -> p ko r", p=P))
                bfks_sb = small.tile([R, C], BF16, tag="bfks", name="bfks_sb")
                nc.sync.dma_start(bfks_sb[:], BfkTs[:, :])
                bfvs_sb = small.tile([R, C], BF16, tag="bfvs", name="bfvs_sb")
                nc.sync.dma_start(bfvs_sb[:], BfvTs[:, :])
            if has_lp:
                ap_sb = small.tile([P, KC, R], BF16, tag="ap", name="ap_sb")
                nc.sync.dma_start(ap_sb[:], ApT.rearrange("(ko p) r -> p ko r", p=P))
                bps_sb = small.tile([R, C], BF16, tag="bps", name="bps_sb")
                nc.sync.dma_start(bps_sb[:], BpTs[:, :])
            if has_bq:
                bq_sb = small.tile([P, KC], F32, tag="bq", name="bq_sb")
                nc.sync.dma_start(bq_sb[:], bq_pp[:, :])
            if has_bfk:
                bfk_sb = small.tile([P, KC], F32, tag="bfk", name="bfk_sb")
                nc.sync.dma_start(bfk_sb[:], bfk_pp[:, :])
            if has_bfv:
                bfv_sb = small.tile([1, C], BF16, tag="bfv", name="bfv_sb")
                nc.sync.dma_start(bfv_sb[:], bfv_row[:, :])
            if has_bp:
                bp_sb = small.tile([1, C], BF16, tag="bp", name="bp_sb")
                nc.sync.dma_start(bp_sb[:], bp_row[:, :])
            ones1 = None
            if has_bfv or has_bp:
                ones1 = small.tile([1, P], BF16, tag="ones1", name="ones1")
                nc.scalar.activation(
                    ones1[:], mask_sb[0:1, 0:P],
                    mybir.ActivationFunctionType.Copy, bias=1.0, scale=0.0,
                )

            # ---- LoRA u-vector for kv (needs only fT) --------------------------
            if has_lf:
                ufs = psA.tile([P, S], F32, tag="mm", name="uf_ps")
                for k in range(KC):
                    nc.tensor.matmul(
                        ufs[:R, :], af_sb[:, k, :], fTs[k][:],
                        start=(k == 0), stop=(k == KC - 1),
                    )
                uf_sb = small.tile([R, S], BF16, tag="uf", name="uf_sb")
                nc.scalar.copy(uf_sb[:], ufs[:R, :])

            # ---- k projection: kT [C, S] ---------------------------------------
            kTs = [small.tile([P, S], BF16, tag=f"kT{m}", name=f"kT{m}") for m in range(KC)]
            for m in range(KC):
                ps = psA.tile([P, S], F32, tag="mm", name=f"k_ps{m}")
                for k in range(KC):
                    nc.tensor.matmul(
                        ps[:], wfk[k][:, mP(m)], fTs[k][:],
                        start=(k == 0), stop=(k == KC - 1 and not has_lf),
                    )
                if has_lf:
                    nc.tensor.matmul(
                        ps[:], bfks_sb[:, mP(m)], uf_sb[:],
                        start=False, stop=True,
                    )
                if has_bfk:
                    nc.scalar.activation(
                        kTs[m][:], ps[:], mybir.ActivationFunctionType.Identity,
                        bias=bfk_sb[:, m:m + 1], scale=1.0,
                    )
                elif m % 2 == 0:
                    nc.scalar.copy(kTs[m][:], ps[:])
                else:
                    nc.vector.tensor_copy(kTs[m][:], ps[:])

            # ---- v projection: v_sb[s2] [128 s, C] -----------------------------
            v_sb = [
                small.tile([P, C], BF16, tag=f"v{s2}", name=f"v{s2}")
                for s2 in range(2)
            ]
            for s2 in range(2):
                for ch in range(2):
                    ps = psA.tile([P, 512], F32, tag="mm", name=f"v_ps{s2}_{ch}")
                    nmm = KC + (1 if has_lf else 0) + (1 if has_bfv else 0)
                    i = 0
                    for k in range(KC):
                        i += 1
                        nc.tensor.matmul(
                            ps[:], fTs[k][:, s2 * P:(s2 + 1) * P],
                            wfv[k][:, c512(ch)],
                            start=(i == 1), stop=(i == nmm),
                        )
                    if has_lf:
                        i += 1
                        nc.tensor.matmul(
                            ps[:], uf_sb[:, s2 * P:(s2 + 1) * P],
                            bfvs_sb[:, c512(ch)], start=False, stop=(i == nmm),
                        )
                    if has_bfv:
                        i += 1
                        nc.tensor.matmul(
                            ps[:], ones1[:], bfv_sb[:, c512(ch)],
                            start=False, stop=(i == nmm),
                        )
                    if (s2 + ch) % 2 == 0:
                        nc.scalar.copy(v_sb[s2][:, c512(ch)], ps[:])
                    else:
                        nc.vector.tensor_copy(v_sb[s2][:, c512(ch)], ps[:])

            # ---- LoRA u-vector for q (needs xT) --------------------------------
            if has_lq:
                uq_sb = small.tile([R, T], BF16, tag="uq", name="uq_sb")
                for ch in range(2):
                    ups = psA.tile([P, 512], F32, tag="mm", name=f"uq_ps{ch}")
                    for k in range(KC):
                        nc.tensor.matmul(
                            ups[:R, :], aq_sb[:, k, :], xTs[k][:, c512(ch)],
                            start=(k == 0), stop=(k == KC - 1),
                        )
                    nc.scalar.copy(uq_sb[:, c512(ch)], ups[:R, :])

            # ---- interleaved per-half pipeline ---------------------------------
            qTs = [qpool.tile([P, T], BF16, tag="qT", name=f"qT{m}") for m in range(MT)]
            yTr = [ypool.tile([P, T], BF16, tag="y", name=f"yTr{p}") for p in range(KC)]
            rz = [psR.tile([H, HW], F32, tag="rz", name=f"rz{hf}") for hf in range(NHF)]
            recs = [
                small.tile([H, HW], F32R, tag=f"rec{hf}", name=f"rec{hf}")
                for hf in range(NHF)
            ]
            up_sb = None
            if has_lp:
                up_sb = small.tile([R, T], BF16, tag="up", name="up_sb")

            def hslc(hf):
                return slice(hf * HW, (hf + 1) * HW)

            def qproj(m, hf):
                # qT[mP(m), t-half]
                ps = psY.tile([P, HW], F32, tag="y", name=f"q_ps{m}_{hf}")
                for k in range(KC):
                    nc.tensor.matmul(
                        ps[:], wq[k][:, mP(m)], xTs[k][:, hslc(hf)],
                        start=(k == 0), stop=(k == KC - 1 and not has_lq),
                    )
                if has_lq:
                    nc.tensor.matmul(
                        ps[:], bqs_sb[:, mP(m)], uq_sb[:, hslc(hf)],
                        start=False, stop=True,
                    )
                if has_bq:
                    nc.scalar.activation(
                        qTs[m][:, hslc(hf)], ps[:],
                        mybir.ActivationFunctionType.Identity,
                        bias=bq_sb[:, m:m + 1], scale=1.0,
                    )
                else:
                    nc.vector.tensor_copy(qTs[m][:, hslc(hf)], ps[:])

            es_head = [None] * 4   # head-slots in flight
            psy_cur = [None]       # live pair-packed attv psum tile

            def scores_exp(h, hf):
                # head h on t-half hf: scores -> exp(*mask) into SBUF
                p, off = h // 2, (h % 2) * D
                kt_h = kTs[p][off:off + D, :]
                qt_h = qTs[p][off:off + D, hslc(hf)]
                es2 = []
                for s2 in range(2):
                    e = expp.tile([P, HW], BF16, tag="exp", name=f"e{h}_{hf}_{s2}")
                    es2.append(e)
                    ps = psA.tile([P, HW], F32, tag="mm", name=f"s_ps{h}_{hf}_{s2}")
                    nc.tensor.matmul(
                        ps[:], kt_h[:, s2 * P:(s2 + 1) * P], qt_h[:],
                        start=True, stop=True,
                    )
                    if hf == 0 and s2 == 1:
                        # t in [0,128) is fully masked for s in [128,256):
                        # memset the dead block, exp only the live part.
                        nc.gpsimd.memset(e[:, 0:P], 0.0)
                        nc.scalar.activation(
                            e[:, P:HW], ps[:, P:HW],
                            mybir.ActivationFunctionType.Exp, scale=0.125,
                        )
                        nc.vector.tensor_mul(
                            e[:, P:2 * P], e[:, P:2 * P],
                            mask_sb[:, 2 * P:3 * P],
                        )
                    else:
                        nc.scalar.activation(
                            e[:], ps[:],
                            mybir.ActivationFunctionType.Exp, scale=0.125,
                        )
                        if hf == 0 and s2 == 0:
                            nc.vector.tensor_mul(
                                e[:, 0:P], e[:, 0:P], mask_sb[:, 0:P],
                            )
                es_head[h % 4] = es2

            def rz_attv(h, hf):
                # consume head h's exp tiles: rowsum matmuls + pair-packed attv
                es2 = es_head[h % 4]
                p, off = h // 2, (h % 2) * D
                if off == 0:
                    psy_cur[0] = psY.tile([P, HW], F32, tag="y", name=f"y_ps{p}_{hf}")
                psy = psy_cur[0]
                for s2 in range(2):
                    # rz[hf][h, :] += sum_s es (indicator matmul)
                    nc.tensor.matmul(
                        rz[hf][:], hsel_sb[:, h, :], es2[s2][:],
                        start=(h == 0 and s2 == 0),
                        stop=(h == H - 1 and s2 == 1),
                    )
                for s2 in range(2):
                    # attv: even head -> psum rows 0:64, odd head -> 64:128
                    nc.tensor.matmul(
                        psy[off:off + D, :],
                        v_sb[s2][:, p * P + off:p * P + off + D],
                        es2[s2][:], start=(s2 == 0), stop=(s2 == 1),
                    )
                if off == D:
                    nc.vector.tensor_copy(yTr[p][:, hslc(hf)], psy[:])

            def normalize(hf):
                rzf = small.tile([H, HW], F32, tag=f"rzf{hf}", name=f"rzf{hf}")
                nc.vector.tensor_copy(rzf[:], rz[hf][:])
                recf = small.tile([H, HW], F32, tag=f"recf{hf}", name=f"recf{hf}")
                nc.vector.reciprocal_approx_fast(recf[:], rzf[:])
                nc.vector.tensor_copy(recs[hf][:], recf[:])
                for p in range(KC):
                    rb = psA.tile([P, HW], F32, tag="mm", name=f"rb{p}_{hf}")
                    nc.tensor.matmul(
                        rb[:], esel_sb[:, mP(p)], recs[hf][:],
                        start=True, stop=True,
                    )
                    nc.vector.tensor_mul(
                        yTr[p][:, hslc(hf)], yTr[p][:, hslc(hf)], rb[:]
                    )

            def oproj_m(m, hf):
                # out rows mP(m) (t in half hf); LoRA up-vector chunk on demand
                if has_lp:
                    upsd = psA.tile([P, P], F32, tag="mm", name=f"up_ps{m}")
                    for k in range(KC):
                        nc.tensor.matmul(
                            upsd[:R, :], ap_sb[:, k, :], yTr[k][:, mP(m)],
                            start=(k == 0), stop=(k == KC - 1),
                        )
                    nc.scalar.copy(up_sb[:, mP(m)], upsd[:R, :])
                for ch in range(2):
                    ps = psA.tile([P, 512], F32, tag="mm", name=f"o_ps{m}_{ch}")
                    nmm = KC + (1 if has_lp else 0) + (1 if has_bp else 0)
                    i = 0
                    for k in range(KC):
                        i += 1
                        nc.tensor.matmul(
                            ps[:], yTr[k][:, mP(m)], wp[k][:, c512(ch)],
                            start=(i == 1), stop=(i == nmm),
                        )
                    if has_lp:
                        i += 1
                        nc.tensor.matmul(
                            ps[:], up_sb[:, mP(m)], bps_sb[:, c512(ch)],
                            start=False, stop=(i == nmm),
                        )
                    if has_bp:
                        i += 1
                        nc.tensor.matmul(
                            ps[:], ones1[:], bp_sb[:, c512(ch)],
                            start=False, stop=(i == nmm),
                        )
                    ost = ostg.tile([P, 512], F32, tag="ostage", name=f"ost{m}_{ch}")
                    nc.scalar.copy(ost[:], ps[:])
                    if hf == 0:
                        nc.scalar.dma_start(out[mP(m), c512(ch)], ost[:])
                    else:
                        nc.sync.dma_start(out[mP(m), c512(ch)], ost[:])

            # Software pipeline (head-granular steps): qproj two pairs ahead;
            # scores+exp one head ahead of rz+attv (hides the scalar exp
            # latency behind a ~2.5-step PSUM ring); oproj of half 0
            # interleaved into half 1's head loop.
            OPROJ_AT = {3: 0, 7: 1, 11: 2, 14: 3}
            for hf in range(NHF):
                for h in range(H):
                    if hf == 0 and h == 0:
                        qproj(0, 0)
                        qproj(1, 0)
                    if h % 2 == 0:
                        la = h // 2 + 2
                        if la < KC:
                            qproj(la, hf)
                        elif hf < NHF - 1:
                            qproj(la - KC, hf + 1)
                    if h >= 1:
                        rz_attv(h - 1, hf)
                    scores_exp(h, hf)
                    if hf == 1 and h in OPROJ_AT:
                        # interleave half-0 output tiles under half-1 attention
                        oproj_m(OPROJ_AT[h], 0)
                rz_attv(H - 1, hf)
                normalize(hf)
            for m in range(4, MT):
                oproj_m(m, 1)

    nc.finalize()
    return nc


def _bf(a):
    return np.ascontiguousarray(np.asarray(a, np.float32).astype(NPBF16))


def _host_prep(x, feature, Wq, bq, Aq, Bq, Wf, bf, Af, Bf, Wp, bp, Ap, Bp):
    f32 = np.float32
    flags = (
        bool(np.any(Bq)), bool(np.any(Bf)), bool(np.any(Bp)),
        bool(np.any(bq)), bool(np.any(bf[:C])), bool(np.any(bf[C:])),
        bool(np.any(bp)),
    )
    shared = {
        "WqT": _bf(np.asarray(Wq, f32).T),
        "WfkT": _bf(np.asarray(Wf[:C], f32).T),
        "WfvT": _bf(np.asarray(Wf[C:], f32).T),
        "WpT": _bf(np.asarray(Wp, f32).T),
    }
    i = np.arange(P)[:, None]
    j = np.arange(384)[None, :]
    m0 = (j[:, :P] >= i).astype(f32)
    m1 = ((j[:, P:384] - P) >= (P + i)).astype(f32)
    shared["mask"] = _bf(np.concatenate([m0, m1], axis=1))
    hsel = np.arange(H)[:, None]
    col = np.arange(C)[None, :]
    shared["Esel"] = np.ascontiguousarray((hsel == col // D).astype(f32))
    hh = np.arange(H)[:, None]
    jj = np.arange(H)[None, :]
    ind = (hh == jj).astype(f32)  # [H, H] identity; column h selected per head
    shared["Hsel"] = _bf(np.broadcast_to(ind[None, :, :], (P, H, H)).reshape(P, H * H))
    has_lq, has_lf, has_lp, has_bq, has_bfk, has_bfv, has_bp = flags
    if has_lq:
        shared["AqT"] = _bf(np.asarray(Aq, f32).T)
        shared["BqTs"] = _bf(np.asarray(Bq, f32).T * SCALING)
    if has_lf:
        shared["AfT"] = _bf(np.asarray(Af, f32).T)
        shared["BfkTs"] = _bf(np.asarray(Bf[:C], f32).T * SCALING)
        shared["BfvTs"] = _bf(np.asarray(Bf[C:], f32).T * SCALING)
    if has_lp:
        shared["ApT"] = _bf(np.asarray(Ap, f32).T)
        shared["BpTs"] = _bf(np.asarray(Bp, f32).T * SCALING)
    if has_bq:
        shared["bq_pp"] = np.ascontiguousarray(np.asarray(bq, f32).reshape(KC, P).T)
    if has_bfk:
        shared["bfk_pp"] = np.ascontiguousarray(np.asarray(bf[:C], f32).reshape(KC, P).T)
    if has_bfv:
        shared["bfv_row"] = _bf(np.asarray(bf[C:], f32).reshape(1, C))
    if has_bp:
        shared["bp_row"] = _bf(np.asarray(bp, f32).reshape(1, C))

    in_maps = []
    for b in range(B):
        m = dict(shared)
        m["xT"] = _bf(np.asarray(x[b], f32).T)
        m["fT"] = _bf(np.asarray(feature[b], f32).T)
        in_maps.append(m)
    return flags, in_maps


def _run(inputs, trace=False, **spmd_kwargs):
    flags, in_maps = _host_prep(**inputs)
    nc = _nc_cache.get(flags)
    if nc is None:
        nc = _build(flags)
        _nc_cache[flags] = nc
    res = run_bass_kernel_spmd(
        nc, in_maps, core_ids=list(range(B)), trace=trace, **spmd_kwargs
    )
    out = np.stack([res.results[b]["out"] for b in range(B)], axis=0)
    return out, res


def kernel(**inputs):
    out, _ = _run(inputs, trace=False)
    return out


# revision 35
# speedup vs baseline: 1.1045x; 1.1045x over previous
"""Trainium2 Bass kernel for CrossAttention with LoRA.

Strategy: data-parallel over batch (B=8 -> 8 NeuronCores, one batch element
per core). No collectives. Per-core compute is a fully fused cross-attention,
restructured (vs the previous version) around three measured bottlenecks:
DVE (vector) saturation, a long serial tail, and rowsum row-copies.

  kT [C,S] = (Wf[:C].T row-tile col-slices) @ fT
  v  [S,C] = fT col-slices as lhsT @ Wf[C:].T          (natural layout)
  qT [C,T] = (Wq.T row-tile col-slices) @ xT           (x passed pre-transposed)
  per half hf (t in [hf*512,(hf+1)*512)):
    per head h: s[S,Thalf] = kT_h.T-slices @ qT_h      (K=D=64)
                e = exp(s/8) (* causal mask, first half only)
                rowsums accumulate into rz[16,512] via indicator matmuls
                  (lhsT = ones-column-h [128,16]) -- no [1,T] row copies
    attv: pair-packed psum [128,512] (even head rows 0:64, odd 64:128 via
                  tile_position), ONE drain per (pair, half)
    normalize: reciprocal_approx_fast + f32r selection-matrix broadcast
                  matmul, yTr *= rb (DVE/GpSimd split)
    oproj m-tiles inside this half start as soon as the half is normalized,
                  overlapping the other half's attention.
Engine budget: exp stream on Scalar; PSUM drains split DVE/GpSimd; ost
(out staging) on Scalar/GpSimd; output stores split across both HWDGE
queues. Weight/input DMAs ordered by first use (wp last).
LoRA terms (rank 16) and biases fold into the same PSUM accumulation groups;
they are skipped at trace time when the corresponding host arrays are zero
(true for loralib-initialized B matrices and zero biases).
"""

import ml_dtypes
import numpy as np

import concourse.bass as bass  # noqa: F401  (bass types via bacc)
import concourse.mybir as mybir
import concourse.tile as tile
from concourse import bacc
from concourse.bass_utils import run_bass_kernel_spmd

B, T, S, C, H, D, R = 8, 1024, 256, 1024, 16, 64, 16
SCALING = 1.0 / 16.0
W8SC = 64.0
ESC = 0.125 / W8SC
P = 128
KC = C // P  # 8 k-tiles over the embedding dim
MT = T // P  # 8 tiles over T
NHF = 2      # two T-halves of 512
HW = 512     # half width
F32 = mybir.dt.float32
F32R = mybir.dt.float32r
BF16 = mybir.dt.bfloat16
FP8 = mybir.dt.float8e4
DR = mybir.MatmulPerfMode.DoubleRow
K2C = KC // 2  # 4 k-pair tiles for fp8 DoubleRow
NPBF16 = ml_dtypes.bfloat16
NPFP8 = ml_dtypes.float8_e4m3fn

_nc_cache: dict = {}


def _build(flags):
    has_lq, has_lf, has_lp, has_bq, has_bfk, has_bfv, has_bp = flags
    nc = bacc.Bacc("TRN2", target_bir_lowering=False, debug=False)

    xT8 = nc.declare_dram_parameter("xT8", [C, T], FP8, isOutput=False)
    fT = nc.declare_dram_parameter("fT", [C, S], BF16, isOutput=False)
    Wq8 = nc.declare_dram_parameter("Wq8", [C, C], FP8, isOutput=False)
    WfkT = nc.declare_dram_parameter("WfkT", [C, C], BF16, isOutput=False)
    WfvT = nc.declare_dram_parameter("WfvT", [C, C], BF16, isOutput=False)
    WpT = nc.declare_dram_parameter("WpT", [C, C], BF16, isOutput=False)
    mask = nc.declare_dram_parameter("mask", [P, 384], BF16, isOutput=False)
    Esel = nc.declare_dram_parameter("Esel", [H, C], BF16, isOutput=False)
    Hsel = nc.declare_dram_parameter("Hsel", [P, H * H], BF16, isOutput=False)
    if has_lq:
        AqT = nc.declare_dram_parameter("AqT", [C, R], BF16, isOutput=False)
        BqTs = nc.declare_dram_parameter("BqTs", [R, C], BF16, isOutput=False)
    if has_lf:
        AfT = nc.declare_dram_parameter("AfT", [C, R], BF16, isOutput=False)
        BfkTs = nc.declare_dram_parameter("BfkTs", [R, C], BF16, isOutput=False)
        BfvTs = nc.declare_dram_parameter("BfvTs", [R, C], BF16, isOutput=False)
    if has_lp:
        ApT = nc.declare_dram_parameter("ApT", [C, R], BF16, isOutput=False)
        BpTs = nc.declare_dram_parameter("BpTs", [R, C], BF16, isOutput=False)
    if has_bq:
        bq_pp = nc.declare_dram_parameter("bq_pp", [P, KC], F32, isOutput=False)
    if has_bfk:
        bfk_pp = nc.declare_dram_parameter("bfk_pp", [P, KC], F32, isOutput=False)
    if has_bfv:
        bfv_row = nc.declare_dram_parameter("bfv_row", [1, C], BF16, isOutput=False)
    if has_bp:
        bp_row = nc.declare_dram_parameter("bp_row", [1, C], BF16, isOutput=False)
    out = nc.declare_dram_parameter("out", [T, C], F32, isOutput=True)

    # 4D k-pair views (DoubleRow) and 3D row-tile views for tiled DMA
    xT84 = xT8.rearrange("(k2 two p) t -> k2 p two t", two=2, p=P)
    fT3 = fT.rearrange("(ko p) s -> ko p s", p=P)
    Wq84 = Wq8.rearrange("(k2 two p) c -> k2 p two c", two=2, p=P)
    WfkT3 = WfkT.rearrange("(ko p) c -> ko p c", p=P)
    WfvT3 = WfvT.rearrange("(ko p) c -> ko p c", p=P)
    WpT3 = WpT.rearrange("(ko p) c -> ko p c", p=P)

    def c512(i):
        return slice(i * 512, (i + 1) * 512)

    def mP(m):
        return slice(m * P, (m + 1) * P)

    with tile.TileContext(nc) as tc:
        with (
            tc.tile_pool(name="big", bufs=8) as big,      # xT tiles
            tc.tile_pool(name="ypool", bufs=8) as ypool,  # yTr tiles
            tc.tile_pool(name="qpool", bufs=8) as qpool,  # qT tiles
            tc.tile_pool(name="wts", bufs=32) as wts,     # wfk/wfv/wq/wp row tiles
            tc.tile_pool(name="small", bufs=1) as small,  # long-lived small tiles
            tc.tile_pool(name="expp", bufs=12) as expp,    # per-head exp tiles
            tc.tile_pool(name="ostg", bufs=4) as ostg,    # out staging
            tc.tile_pool(name="psA", bufs=5, space="PSUM") as psA,  # score tiles
            tc.tile_pool(name="psY", bufs=2, space="PSUM") as psY,  # qproj + attv
            tc.tile_pool(name="psR", bufs=1, space="PSUM") as psR,  # rowsum tiles
        ):
            # ---- SP queue: mask/hsel first (tiny), then k-proj operands, then
            # fT/wfv halves for vproj, wp last (needed only at oproj) ----------
            mask_sb = small.tile([P, 384], BF16, tag="mask", name="mask_sb")
            nc.sync.dma_start(mask_sb[:], mask[:, :])
            hsel_sb = small.tile([P, H, H], BF16, tag="hsel", name="hsel_sb")
            nc.sync.dma_start(hsel_sb[:], Hsel.rearrange("p (h j) -> p h j", h=H))
            fTs = [small.tile([P, S], BF16, tag=f"fT{k}", name=f"fT{k}") for k in range(KC)]
            for k in range(KC):
                nc.sync.dma_start(fTs[k][:], fT3[k])
            wfk = [wts.tile([P, C], BF16, tag="wts", name=f"wfk{k}") for k in range(KC)]
            for k in range(KC):
                nc.sync.dma_start(wfk[k][:], WfkT3[k])
            wfv = [wts.tile([P, C], BF16, tag="wts", name=f"wfv{k}") for k in range(KC)]
            for k in range(4):
                nc.sync.dma_start(wfv[k][:], WfvT3[k])
            wp = [wts.tile([P, C], BF16, tag="wts", name=f"wp{k}") for k in range(KC)]
            for k in range(KC):
                nc.sync.dma_start(wp[k][:], WpT3[k])
            # ---- Act queue: q-projection operands, wfv tail, then esel ---------
            xT8s = [big.tile([P, 2, T], FP8, tag="big", name=f"xT8{k}") for k in range(K2C)]
            for k in range(K2C):
                nc.scalar.dma_start(xT8s[k][:], xT84[k])
            wq8 = [wts.tile([P, 2, C], FP8, tag="wts", name=f"wq8{k}") for k in range(K2C)]
            for k in range(K2C):
                nc.scalar.dma_start(wq8[k][:], Wq84[k])
            for k in range(4, KC):
                nc.scalar.dma_start(wfv[k][:], WfvT3[k])
            esel_sb = small.tile([H, C], BF16, tag="esel", name="esel_sb")
            nc.scalar.dma_start(esel_sb[:], Esel[:, :])
            # ---- small conditional loads (SP) ----------------------------------
            if has_lq:
                aq_sb = small.tile([P, KC, R], BF16, tag="aq", name="aq_sb")
                nc.sync.dma_start(aq_sb[:], AqT.rearrange("(ko p) r -> p ko r", p=P))
                bqs_sb = small.tile([R, C], BF16, tag="bqs", name="bqs_sb")
                nc.sync.dma_start(bqs_sb[:], BqTs[:, :])
            if has_lf:
                af_sb = small.tile([P, KC, R], BF16, tag="af", name="af_sb")
                nc.sync.dma_start(af_sb[:], AfT.rearrange("(ko p) r -> p ko r", p=P))
                bfks_sb = small.tile([R, C], BF16, tag="bfks", name="bfks_sb")
                nc.sync.dma_start(bfks_sb[:], BfkTs[:, :])
                bfvs_sb = small.tile([R, C], BF16, tag="bfvs", name="bfvs_sb")
                nc.sync.dma_start(bfvs_sb[:], BfvTs[:, :])
            if has_lp:
                ap_sb = small.tile([P, KC, R], BF16, tag="ap", name="ap_sb")
                nc.sync.dma_start(ap_sb[:], ApT.rearrange("(ko p) r -> p ko r", p=P))
                bps_sb = small.tile([R, C], BF16, tag="bps", name="bps_sb")
                nc.sync.dma_start(bps_sb[:], BpTs[:, :])
            if has_bq:
                bq_sb = small.tile([P, KC], F32, tag="bq", name="bq_sb")
                nc.sync.dma_start(bq_sb[:], bq_pp[:, :])
            if has_bfk:
                bfk_sb = small.tile([P, KC], F32, tag="bfk", name="bfk_sb")
                nc.sync.dma_start(bfk_sb[:], bfk_pp[:, :])
            if has_bfv:
                bfv_sb = small.tile([1, C], BF16, tag="bfv", name="bfv_sb")
                nc.sync.dma_start(bfv_sb[:], bfv_row[:, :])
            if has_bp:
                bp_sb = small.tile([1, C], BF16, tag="bp", name="bp_sb")
                nc.sync.dma_start(bp_sb[:], bp_row[:, :])
            ones1 = None
            if has_bfv or has_bp:
                ones1 = small.tile([1, P], BF16, tag="ones1", name="ones1")
                nc.scalar.activation(
                    ones1[:], mask_sb[0:1, 0:P],
                    mybir.ActivationFunctionType.Copy, bias=1.0, scale=0.0,
                )

            # ---- LoRA u-vector for kv (needs only fT) --------------------------
            if has_lf:
                ufs = psA.tile([P, S], F32, tag="mm", name="uf_ps")
                for k in range(KC):
                    nc.tensor.matmul(
                        ufs[:R, :], af_sb[:, k, :], fTs[k][:],
                        start=(k == 0), stop=(k == KC - 1),
                    )
                uf_sb = small.tile([R, S], BF16, tag="uf", name="uf_sb")
                nc.scalar.copy(uf_sb[:], ufs[:R, :])

            # ---- k projection: kT [C, S] ---------------------------------------
            kTs = [small.tile([P, S], BF16, tag=f"kT{m}", name=f"kT{m}") for m in range(KC)]
            for m in range(KC):
                ps = psA.tile([P, S], F32, tag="mm", name=f"k_ps{m}")
                for k in range(KC):
                    nc.tensor.matmul(
                        ps[:], wfk[k][:, mP(m)], fTs[k][:],
                        start=(k == 0), stop=(k == KC - 1 and not has_lf),
                    )
                if has_lf:
                    nc.tensor.matmul(
                        ps[:], bfks_sb[:, mP(m)], uf_sb[:],
                        start=False, stop=True,
                    )
                if has_bfk:
                    nc.scalar.activation(
                        kTs[m][:], ps[:], mybir.ActivationFunctionType.Identity,
                        bias=bfk_sb[:, m:m + 1], scale=1.0,
                    )
                elif m % 2 == 0:
                    nc.scalar.copy(kTs[m][:], ps[:])
                else:
                    nc.vector.tensor_copy(kTs[m][:], ps[:])

            # ---- v projection: v_sb[s2] [128 s, C] -----------------------------
            v_sb = [
                small.tile([P, C], BF16, tag=f"v{s2}", name=f"v{s2}")
                for s2 in range(2)
            ]
            for s2 in range(2):
                for ch in range(2):
                    ps = psA.tile([P, 512], F32, tag="mm", name=f"v_ps{s2}_{ch}")
                    nmm = KC + (1 if has_lf else 0) + (1 if has_bfv else 0)
                    i = 0
                    for k in range(KC):
                        i += 1
                        nc.tensor.matmul(
                            ps[:], fTs[k][:, s2 * P:(s2 + 1) * P],
                            wfv[k][:, c512(ch)],
                            start=(i == 1), stop=(i == nmm),
                        )
                    if has_lf:
                        i += 1
                        nc.tensor.matmul(
                            ps[:], uf_sb[:, s2 * P:(s2 + 1) * P],
                            bfvs_sb[:, c512(ch)], start=False, stop=(i == nmm),
                        )
                    if has_bfv:
                        i += 1
                        nc.tensor.matmul(
                            ps[:], ones1[:], bfv_sb[:, c512(ch)],
                            start=False, stop=(i == nmm),
                        )
                    if (s2 + ch) % 2 == 0:
                        nc.scalar.copy(v_sb[s2][:, c512(ch)], ps[:])
                    else:
                        nc.vector.tensor_copy(v_sb[s2][:, c512(ch)], ps[:])

            # ---- LoRA u-vector for q (needs xT) --------------------------------
            if has_lq:
                uq_sb = small.tile([R, T], BF16, tag="uq", name="uq_sb")
                for ch in range(2):
                    ups = psA.tile([P, 512], F32, tag="mm", name=f"uq_ps{ch}")
                    for k in range(KC):
                        nc.tensor.matmul(
                            ups[:R, :], aq_sb[:, k, :],
                            xT8s[k // 2][:, k % 2, c512(ch)],
                            start=(k == 0), stop=(k == KC - 1),
                        )
                    nc.scalar.copy(uq_sb[:, c512(ch)], ups[:R, :])

            # ---- interleaved per-half pipeline ---------------------------------
            qTs = [qpool.tile([P, T], BF16, tag="qT", name=f"qT{m}") for m in range(MT)]
            yTr = [ypool.tile([P, T], BF16, tag="y", name=f"yTr{p}") for p in range(KC)]
            rz = [psR.tile([H, HW], F32, tag="rz", name=f"rz{hf}") for hf in range(NHF)]
            recs = [
                small.tile([H, HW], BF16, tag=f"rec{hf}", name=f"rec{hf}")
                for hf in range(NHF)
            ]
            up_sb = None
            if has_lp:
                up_sb = small.tile([R, T], BF16, tag="up", name="up_sb")

            def hslc(hf):
                return slice(hf * HW, (hf + 1) * HW)

            def qproj(m, hf):
                # qT[mP(m), t-half], fp8 DoubleRow over k-pairs
                ps = psY.tile([P, HW], F32, tag="y", name=f"q_ps{m}_{hf}")
                for k in range(K2C):
                    nc.tensor.matmul(
                        ps[:], wq8[k][:, :, mP(m)], xT8s[k][:, :, hslc(hf)],
                        start=(k == 0), stop=(k == K2C - 1 and not has_lq),
                        perf_mode=DR,
                    )
                if has_lq:
                    nc.tensor.matmul(
                        ps[:], bqs_sb[:, mP(m)], uq_sb[:, hslc(hf)],
                        start=False, stop=True,
                    )
                if has_bq:
                    nc.scalar.activation(
                        qTs[m][:, hslc(hf)], ps[:],
                        mybir.ActivationFunctionType.Identity,
                        bias=bq_sb[:, m:m + 1], scale=1.0,
                    )
                else:
                    nc.vector.tensor_copy(qTs[m][:, hslc(hf)], ps[:])

            es_head = [None] * 8   # head-slots in flight
            psy_cur = [None]       # live pair-packed attv psum tile

            def scores_exp(h, hf):
                # head h on t-half hf: scores -> exp(*mask) into SBUF
                p, off = h // 2, (h % 2) * D
                kt_h = kTs[p][off:off + D, :]
                qt_h = qTs[p][off:off + D, hslc(hf)]
                es2 = []
                for s2 in range(2):
                    e = expp.tile([P, HW], BF16, tag="exp", name=f"e{h}_{hf}_{s2}")
                    es2.append(e)
                    ps = psA.tile([P, HW], F32, tag="mm", name=f"s_ps{h}_{hf}_{s2}")
                    nc.tensor.matmul(
                        ps[:], kt_h[:, s2 * P:(s2 + 1) * P], qt_h[:],
                        start=True, stop=True,
                    )
                    if hf == 0 and s2 == 1:
                        # t in [0,128) is fully masked for s in [128,256):
                        # memset the dead block, exp only the live part.
                        nc.gpsimd.memset(e[:, 0:P], 0.0)
                        nc.scalar.activation(
                            e[:, P:HW], ps[:, P:HW],
                            mybir.ActivationFunctionType.Exp, scale=ESC,
                        )
                        nc.vector.tensor_mul(
                            e[:, P:2 * P], e[:, P:2 * P],
                            mask_sb[:, 2 * P:3 * P],
                        )
                    else:
                        nc.scalar.activation(
                            e[:], ps[:],
                            mybir.ActivationFunctionType.Exp, scale=ESC,
                        )
                        if hf == 0 and s2 == 0:
                            nc.vector.tensor_mul(
                                e[:, 0:P], e[:, 0:P], mask_sb[:, 0:P],
                            )
                es_head[h % 8] = es2

            def rz_attv(h, hf):
                # consume head h's exp tiles: rowsum matmuls + pair-packed attv
                es2 = es_head[h % 8]
                p, off = h // 2, (h % 2) * D
                if off == 0:
                    psy_cur[0] = psY.tile([P, HW], F32, tag="y", name=f"y_ps{p}_{hf}")
                psy = psy_cur[0]
                for s2 in range(2):
                    # rz[hf][h, :] += sum_s es (indicator matmul)
                    nc.tensor.matmul(
                        rz[hf][:], hsel_sb[:, h, :], es2[s2][:],
                        start=(h == 0 and s2 == 0),
                        stop=(h == H - 1 and s2 == 1),
                    )
                for s2 in range(2):
                    # attv: even head -> psum rows 0:64, odd head -> 64:128
                    nc.tensor.matmul(
                        psy[off:off + D, :],
                        v_sb[s2][:, p * P + off:p * P + off + D],
                        es2[s2][:], start=(s2 == 0), stop=(s2 == 1),
                    )
                if off == D:
                    nc.vector.tensor_copy(yTr[p][:, hslc(hf)], psy[:])

            def normalize(hf):
                rzf = small.tile([H, HW], F32, tag=f"rzf{hf}", name=f"rzf{hf}")
                nc.vector.tensor_copy(rzf[:], rz[hf][:])
                recf = small.tile([H, HW], F32, tag=f"recf{hf}", name=f"recf{hf}")
                nc.vector.reciprocal_approx_fast(recf[:], rzf[:])
                nc.vector.tensor_copy(recs[hf][:], recf[:])
                for p in range(KC):
                    rb = psA.tile([P, HW], F32, tag="mm", name=f"rb{p}_{hf}")
                    nc.tensor.matmul(
                        rb[:], esel_sb[:, mP(p)], recs[hf][:],
                        start=True, stop=True,
                    )
                    nc.vector.tensor_mul(
                        yTr[p][:, hslc(hf)], yTr[p][:, hslc(hf)], rb[:]
                    )

            def oproj_m(m, hf):
                # out rows mP(m) (t in half hf); LoRA up-vector chunk on demand
                if has_lp:
                    upsd = psA.tile([P, P], F32, tag="mm", name=f"up_ps{m}")
                    for k in range(KC):
                        nc.tensor.matmul(
                            upsd[:R, :], ap_sb[:, k, :], yTr[k][:, mP(m)],
                            start=(k == 0), stop=(k == KC - 1),
                        )
                    nc.scalar.copy(up_sb[:, mP(m)], upsd[:R, :])
                for ch in range(2):
                    ps = psA.tile([P, 512], F32, tag="mm", name=f"o_ps{m}_{ch}")
                    nmm = KC + (1 if has_lp else 0) + (1 if has_bp else 0)
                    i = 0
                    for k in range(KC):
                        i += 1
                        nc.tensor.matmul(
                            ps[:], yTr[k][:, mP(m)], wp[k][:, c512(ch)],
                            start=(i == 1), stop=(i == nmm),
                        )
                    if has_lp:
                        i += 1
                        nc.tensor.matmul(
                            ps[:], up_sb[:, mP(m)], bps_sb[:, c512(ch)],
                            start=False, stop=(i == nmm),
                        )
                    if has_bp:
                        i += 1
                        nc.tensor.matmul(
                            ps[:], ones1[:], bp_sb[:, c512(ch)],
                            start=False, stop=(i == nmm),
                        )
                    ost = ostg.tile([P, 512], F32, tag="ostage", name=f"ost{m}_{ch}")
                    nc.scalar.copy(ost[:], ps[:])
                    if hf == 0:
                        nc.scalar.dma_start(out[mP(m), c512(ch)], ost[:])
                    else:
                        nc.sync.dma_start(out[mP(m), c512(ch)], ost[:])

            # Software pipeline (head-granular steps): qproj two pairs ahead;
            # scores+exp one head ahead of rz+attv (hides the scalar exp
            # latency behind a ~2.5-step PSUM ring); oproj of half 0
            # interleaved into half 1's head loop.
            OPROJ_AT = {3: 0, 7: 1, 11: 2, 15: 3}
            LAG = 2
            for hf in range(NHF):
                for h in range(H + LAG):
                    if hf == 0 and h == 0:
                        qproj(0, 0)
                        qproj(1, 0)
                    if h % 2 == 0 and h < H:
                        la = h // 2 + 2
                        if la < KC:
                            qproj(la, hf)
                        elif hf < NHF - 1:
                            qproj(la - KC, hf + 1)
                    if h >= LAG:
                        rz_attv(h - LAG, hf)
                    if h < H:
                        scores_exp(h, hf)
                    if hf == 1 and h in OPROJ_AT:
                        # interleave half-0 output tiles under half-1 attention
                        oproj_m(OPROJ_AT[h], 0)
                normalize(hf)
            for m in range(4, MT):
                oproj_m(m, 1)

    nc.finalize()
    return nc


def _bf(a):
    return np.ascontiguousarray(np.asarray(a, np.float32).astype(NPBF16))


def _f8(a):
    return np.ascontiguousarray(np.asarray(a, np.float32).astype(NPFP8))


def _host_prep(x, feature, Wq, bq, Aq, Bq, Wf, bf, Af, Bf, Wp, bp, Ap, Bp):
    f32 = np.float32
    flags = (
        bool(np.any(Bq)), bool(np.any(Bf)), bool(np.any(Bp)),
        bool(np.any(bq)), bool(np.any(bf[:C])), bool(np.any(bf[C:])),
        bool(np.any(bp)),
    )
    shared = {
        # q/k projection weights in fp8 e4m3, scaled x64 out of the subnormal
        # range; compensated in the on-device exp scale (ESC)
        "Wq8": _f8(np.asarray(Wq, f32).T * W8SC),
        "WfkT": _bf(np.asarray(Wf[:C], f32).T),
        "WfvT": _bf(np.asarray(Wf[C:], f32).T),
        "WpT": _bf(np.asarray(Wp, f32).T),
    }
    i = np.arange(P)[:, None]
    j = np.arange(384)[None, :]
    m0 = (j[:, :P] >= i).astype(f32)
    m1 = ((j[:, P:384] - P) >= (P + i)).astype(f32)
    shared["mask"] = _bf(np.concatenate([m0, m1], axis=1))
    hsel = np.arange(H)[:, None]
    col = np.arange(C)[None, :]
    shared["Esel"] = _bf((hsel == col // D).astype(f32))
    hh = np.arange(H)[:, None]
    jj = np.arange(H)[None, :]
    ind = (hh == jj).astype(f32)  # [H, H] identity; column h selected per head
    shared["Hsel"] = _bf(np.broadcast_to(ind[None, :, :], (P, H, H)).reshape(P, H * H))
    has_lq, has_lf, has_lp, has_bq, has_bfk, has_bfv, has_bp = flags
    if has_lq:
        shared["AqT"] = _bf(np.asarray(Aq, f32).T)
        shared["BqTs"] = _bf(np.asarray(Bq, f32).T * (SCALING * W8SC))
    if has_lf:
        shared["AfT"] = _bf(np.asarray(Af, f32).T)
        shared["BfkTs"] = _bf(np.asarray(Bf[:C], f32).T * SCALING)
        shared["BfvTs"] = _bf(np.asarray(Bf[C:], f32).T * SCALING)
    if has_lp:
        shared["ApT"] = _bf(np.asarray(Ap, f32).T)
        shared["BpTs"] = _bf(np.asarray(Bp, f32).T * SCALING)
    if has_bq:
        shared["bq_pp"] = np.ascontiguousarray(
            np.asarray(bq, f32).reshape(KC, P).T * W8SC
        )
    if has_bfk:
        shared["bfk_pp"] = np.ascontiguousarray(
            np.asarray(bf[:C], f32).reshape(KC, P).T
        )
    if has_bfv:
        shared["bfv_row"] = _bf(np.asarray(bf[C:], f32).reshape(1, C))
    if has_bp:
        shared["bp_row"] = _bf(np.asarray(bp, f32).reshape(1, C))

    in_maps = []
    for b in range(B):
        m = dict(shared)
        xt = np.asarray(x[b], f32).T
        ft = np.asarray(feature[b], f32).T
        m["xT8"] = _f8(xt)
        m["fT"] = _bf(ft)
        in_maps.append(m)
    return flags, in_maps


def _run(inputs, trace=False, **spmd_kwargs):
    flags, in_maps = _host_prep(**inputs)
    nc = _nc_cache.get(flags)
    if nc is None:
        nc = _build(flags)
        _nc_cache[flags] = nc
    res = run_bass_kernel_spmd(
        nc, in_maps, core_ids=list(range(B)), trace=trace, **spmd_kwargs
    )
    out = np.stack([res.results[b]["out"] for b in range(B)], axis=0)
    return out, res


def kernel(**inputs):
    out, _ = _run(inputs, trace=False)
    return out
